# revision 1
# baseline (speedup 1.0000x reference)
"""DynamicConvolution TRN2 Bass kernel.

Problem (per reference):
  x: (32, 128, 64, 64) f32
  attention: pooled = mean(x, HW) -> MLP (relu) -> prompt dot -> softmax over K=8
  agg_w[b] = sum_k alpha[b,k] * kernels_weights[k]  (K=8 banks of (128,128,3,3))
  out[b] = conv2d(x[b], agg_w[b], pad=1) + agg_b[b]   -> (32, 128, 64, 64)

Strategy:
  - Data-parallel over batch: 8 cores x 4 samples.
  - Conv as 9 shifted matmuls (tap-wise) accumulating in PSUM, fp32r (TF32)
    matmuls at 1 col/cycle.  x is zero-padded to 66x66 on host, pre-rounded
    to TF32 (bit-exact with HW fp32r rounding), so every matmul is a full
    (128 x 512) tile: out rows = 8 image rows.
  - Attention MLP batched over the 4 local samples with plain fp32 matmuls
    (tiny).  Softmax on DVE/ACT.  alphas bounced through DRAM to get
    partition-broadcast + k-on-partition layouts.
  - Kernel aggregation: 8 scalar_tensor_tensor ops on DVE per sample
    (fp32), final round to fp32r for the PE.
"""
import sys

sys.path.insert(0, "/opt/trn_rl_repo")

import numpy as np

import concourse.bacc as bacc
import concourse.mybir as mybir
import concourse.tile as tile
from concourse.bass_utils import run_bass_kernel_spmd

# problem dims
B, C, H, W = 32, 128, 64, 64
K, KS = 8, 3
HID = 512
NCORES = 8
BL = B // NCORES          # local batch = 4
HP, WP = H + 2, W + 2     # 66x66 padded
NPIX = HP * WP            # 4356
RCHUNK = 8                # output rows per PSUM chunk
NCHUNK = H // RCHUNK      # 8
F32 = mybir.dt.float32
F32R = mybir.dt.float32r


def round_tf32(a: np.ndarray) -> np.ndarray:
    """Round-to-nearest-even to TF32 (10-bit mantissa) — matches HW fp32r."""
    a = np.ascontiguousarray(a, dtype=np.float32)
    u = a.view(np.uint32).astype(np.uint64)
    r = (u + 0xFFF + ((u >> 13) & 1)) & ~np.uint64(0x1FFF)
    return r.astype(np.uint32).view(np.float32)


def build(timing_chain: bool = False, probe_skip=()):
    """probe_skip: analysis-only knobs ('mlp', 'agg', 'reduce') that stub out
    pipeline stages so TimelineSim can attribute predicted time."""
    nc = bacc.Bacc("TRN2", target_bir_lowering=False, debug=False)

    if timing_chain:
        # unused input whose only purpose is to let a timing harness chain
        # iteration i's output into iteration i+1 (forces serial execution)
        nc.dram_tensor("chain", [BL, C, H * W], F32, kind="ExternalInput")
    xp = nc.dram_tensor("xp", [BL, C, NPIX], F32R, kind="ExternalInput")
    w1t = nc.dram_tensor("w1t", [C, HID], F32, kind="ExternalInput")
    b1c = nc.dram_tensor("b1c", [C, 4], F32, kind="ExternalInput")
    w2t = nc.dram_tensor("w2t", [C, 4, HID], F32, kind="ExternalInput")
    b2c = nc.dram_tensor("b2c", [C, 4], F32, kind="ExternalInput")
    pt = nc.dram_tensor("pt", [C, 4, K], F32, kind="ExternalInput")
    kb = nc.dram_tensor("kb", [K, C], F32, kind="ExternalInput")
    kw = nc.dram_tensor("kw", [C, K, KS * KS, C], F32, kind="ExternalInput")
    out = nc.dram_tensor("out", [BL, C, H * W], F32, kind="ExternalOutput")
    alpha_dram = nc.dram_tensor("alpha_scratch", [BL, K], F32)

    with tile.TileContext(nc) as tc:
        with (
            tc.tile_pool(name="singles", bufs=1) as singles,
            tc.tile_pool(name="xpool", bufs=BL) as xpool,
            tc.tile_pool(name="opool", bufs=2) as opool,
            tc.tile_pool(name="aggpool", bufs=2) as aggpool,
            tc.tile_pool(name="scr", bufs=1) as scr,
            tc.tile_pool(name="mlpp", bufs=2, space="PSUM") as mlpp,
            tc.tile_pool(
                name="convp", bufs=8 if "psum8" in probe_skip else 4, space="PSUM"
            ) as convp,
        ):
            # ---- load x first (padded, fp32r): pooled reduces gate the MLP ----
            x_sb = []
            for s in range(BL):
                xt = xpool.tile([C, HP, WP], F32R, tag="x")
                nc.sync.dma_start(
                    out=xt, in_=xp.ap()[s].rearrange("p (a b) -> p a b", a=HP)
                )
                x_sb.append(xt)

            # ---- load weights ----
            w1t_sb = singles.tile([C, HID], F32)
            nc.sync.dma_start(out=w1t_sb, in_=w1t.ap())
            b1_sb = singles.tile([C, 4], F32)
            nc.sync.dma_start(out=b1_sb, in_=b1c.ap())
            w2t_sb = singles.tile([C, 4, HID], F32)
            nc.sync.dma_start(out=w2t_sb, in_=w2t.ap())
            b2_sb = singles.tile([C, 4], F32)
            nc.sync.dma_start(out=b2_sb, in_=b2c.ap())
            pt_sb = singles.tile([C, 4, K], F32)
            nc.sync.dma_start(out=pt_sb, in_=pt.ap())
            kb_sb = singles.tile([K, C], F32)
            nc.sync.dma_start(out=kb_sb, in_=kb.ap())
            # kernel bank split per-k so aggregation isn't gated on one 4.7MB DMA
            kw_sb = singles.tile([C, K, KS * KS, C], F32)
            for k in range(K):
                nc.sync.dma_start(out=kw_sb[:, k], in_=kw.ap()[:, k])

            # ---- pooled sums (mean folded into relu scale later) ----
            # split across DVE and ACT so the 4 reduces serialize half as long
            pooled = singles.tile([C, BL], F32)
            junk = singles.tile([C, NPIX], F32)
            if "reduce" in probe_skip:
                nc.vector.memset(pooled, 1.0)
            else:
                for s in range(BL):
                    if s % 2 == 0:
                        nc.vector.tensor_reduce(
                            pooled[:, s : s + 1],
                            x_sb[s].bitcast(F32),
                            axis=mybir.AxisListType.XY,
                            op=mybir.AluOpType.add,
                        )
                    else:
                        nc.scalar.activation(
                            junk,
                            x_sb[s].bitcast(F32).rearrange("p a b -> p (a b)"),
                            mybir.ActivationFunctionType.Copy,
                            accum_out=pooled[:, s : s + 1],
                        )

            # ---- attention MLP in two 2-sample pipelines: samples 0-1 reach
            # alphas (and start convs) without waiting for samples 2-3 ----
            skip_mlp = "mlp" in probe_skip
            h_sb = singles.tile([C, 4, BL], F32)
            s_sb = singles.tile([C, 4, BL], F32)
            alpha_bc = singles.tile([C, BL, K], F32)
            alpha_k8 = singles.tile([K, BL], F32)
            aggb_sb = singles.tile([C, BL], F32)
            if skip_mlp:
                nc.vector.memset(alpha_bc, 0.125)
                nc.vector.memset(alpha_k8, 0.125)
                nc.vector.memset(aggb_sb, 0.0)
            for pr in [] if skip_mlp else range(2):
                sl = slice(2 * pr, 2 * pr + 2)
                ps_h = mlpp.tile([C, 4, 2], F32, tag="ps_mlp")
                for c in range(4):
                    nc.tensor.matmul(
                        ps_h[:, c, :], w1t_sb[:, 128 * c : 128 * (c + 1)],
                        pooled[:, sl], start=True, stop=True,
                    )
                    nc.scalar.activation(
                        h_sb[:, c, sl], ps_h[:, c, :],
                        mybir.ActivationFunctionType.Relu,
                        bias=b1_sb[:, c : c + 1], scale=1.0 / (H * W),
                    )
                ps_s = mlpp.tile([C, 4, 2], F32, tag="ps_mlp")
                for c2 in range(4):
                    for c in range(4):
                        nc.tensor.matmul(
                            ps_s[:, c2, :],
                            w2t_sb[:, c, 128 * c2 : 128 * (c2 + 1)],
                            h_sb[:, c, sl],
                            start=(c == 0), stop=(c == 3),
                        )
                    nc.scalar.activation(
                        s_sb[:, c2, sl], ps_s[:, c2, :],
                        mybir.ActivationFunctionType.Identity,
                        bias=b2_sb[:, c2 : c2 + 1],
                    )
                ps_sc = mlpp.tile([2, K], F32, tag="ps_sm")
                for c2 in range(4):
                    nc.tensor.matmul(
                        ps_sc, s_sb[:, c2, sl], pt_sb[:, c2, :],
                        start=(c2 == 0), stop=(c2 == 3),
                    )
                negmx = scr.tile([2, 1], F32, tag="negmx")
                nc.vector.tensor_reduce(
                    negmx, ps_sc, axis=mybir.AxisListType.X,
                    op=mybir.AluOpType.max, negate=True,
                )
                ex = scr.tile([2, K], F32, tag="ex")
                nc.scalar.activation(
                    ex, ps_sc, mybir.ActivationFunctionType.Exp, bias=negmx,
                )
                sm = scr.tile([2, 1], F32, tag="sm")
                nc.vector.tensor_reduce(
                    sm, ex, axis=mybir.AxisListType.X, op=mybir.AluOpType.add
                )
                rsm = scr.tile([2, 1], F32, tag="rsm")
                nc.vector.reciprocal(rsm, sm)
                alphas = scr.tile([2, K], F32, tag="alphas")
                nc.vector.tensor_scalar_mul(alphas, ex, rsm)

                nc.sync.dma_start(out=alpha_dram.ap()[sl], in_=alphas)
                nc.sync.dma_start(
                    out=alpha_bc[:, sl, :],
                    in_=alpha_dram.ap()[sl].rearrange("b k -> (b k)").unsqueeze(0)
                    .to_broadcast((C, 2 * K))
                    .rearrange("p (b k) -> p b k", b=2),
                )
                nc.sync.dma_start(
                    out=alpha_k8[:, sl],
                    in_=alpha_dram.ap()[sl].rearrange("b k -> k b"),
                )
                ps_ab = mlpp.tile([C, 2], F32, tag="ps_sm")
                nc.tensor.matmul(ps_ab, kb_sb, alpha_k8[:, sl], start=True, stop=True)
                nc.scalar.copy(aggb_sb[:, sl], ps_ab)

            # ---- per sample: aggregate kernel bank, conv, bias, store ----
            taps = [(ti, tj) for ti in range(KS) for tj in range(KS)]
            for s in range(BL):
                if "agg" in probe_skip:
                    aggw = aggpool.tile([C, KS * KS, C], F32R, tag="aggw")
                    nc.vector.tensor_copy(aggw, kw_sb[:, 0])
                    o_sb = opool.tile([C, H, W], F32, tag="out")
                    for chunk in range(NCHUNK):
                        h0 = chunk * RCHUNK
                        ps_c = convp.tile([C, RCHUNK, W], F32, tag="ps_c")
                        for t, (ti, tj) in enumerate(taps):
                            nc.tensor.matmul(
                                ps_c, aggw[:, t, :],
                                x_sb[s][:, h0 + ti : h0 + ti + RCHUNK, tj : tj + W],
                                start=(t == 0), stop=(t == KS * KS - 1),
                            )
                        if "evict_dve" in probe_skip:
                            nc.vector.tensor_scalar_add(
                                o_sb[:, h0 : h0 + RCHUNK, :], ps_c,
                                aggb_sb[:, s : s + 1],
                            )
                        else:
                            nc.scalar.activation(
                                o_sb[:, h0 : h0 + RCHUNK, :], ps_c,
                                mybir.ActivationFunctionType.Identity,
                                bias=aggb_sb[:, s : s + 1],
                            )
                    nc.sync.dma_start(
                        out=out.ap()[s], in_=o_sb.rearrange("p a b -> p (a b)")
                    )
                    continue
                # weighted sum of 8 kernel banks on DVE
                sA = aggpool.tile([C, KS * KS, C], F32, tag="aggA")
                sB = aggpool.tile([C, KS * KS, C], F32, tag="aggB")
                pp = [sA, sB]
                nc.vector.tensor_scalar_mul(
                    sA, kw_sb[:, 0], alpha_bc[:, s, 0:1]
                )
                for k in range(1, K - 1):
                    nc.vector.scalar_tensor_tensor(
                        pp[k % 2], kw_sb[:, k], alpha_bc[:, s, k : k + 1],
                        pp[(k + 1) % 2],
                        op0=mybir.AluOpType.mult, op1=mybir.AluOpType.add,
                    )
                aggw = aggpool.tile([C, KS * KS, C], F32R, tag="aggw")
                nc.vector.scalar_tensor_tensor(
                    aggw, kw_sb[:, K - 1], alpha_bc[:, s, K - 1 : K],
                    pp[(K - 2) % 2],
                    op0=mybir.AluOpType.mult, op1=mybir.AluOpType.add,
                )

                o_sb = opool.tile([C, H, W], F32, tag="out")
                for chunk in range(NCHUNK):
                    h0 = chunk * RCHUNK
                    ps_c = convp.tile([C, RCHUNK, W], F32, tag="ps_c")
                    for t, (ti, tj) in enumerate(taps):
                        nc.tensor.matmul(
                            ps_c,
                            aggw[:, t, :],
                            x_sb[s][:, h0 + ti : h0 + ti + RCHUNK, tj : tj + W],
                            start=(t == 0), stop=(t == KS * KS - 1),
                        )
                    nc.scalar.activation(
                        o_sb[:, h0 : h0 + RCHUNK, :], ps_c,
                        mybir.ActivationFunctionType.Identity,
                        bias=aggb_sb[:, s : s + 1],
                    )
                nc.sync.dma_start(
                    out=out.ap()[s], in_=o_sb.rearrange("p a b -> p (a b)")
                )

    nc.compile()
    return nc


_NC = None


def _get_nc():
    global _NC
    if _NC is None:
        _NC = build()
    return _NC


def prep_inputs(x, prompt_param, w1, b1, w2, b2, kernels_weights, kernels_bias):
    """Host-side layout transforms -> per-core in_maps."""
    x = np.asarray(x, np.float32)
    prompt = np.asarray(prompt_param, np.float32)[0]          # (K, HID)
    w1 = np.asarray(w1, np.float32)
    b1 = np.asarray(b1, np.float32)
    w2 = np.asarray(w2, np.float32)
    b2 = np.asarray(b2, np.float32)
    kwt = np.asarray(kernels_weights, np.float32)             # (K, C, C, 3, 3)
    kbt = np.asarray(kernels_bias, np.float32)                # (K, C)

    w1t = np.ascontiguousarray(w1.T)                          # (C, HID)
    b1c = np.ascontiguousarray(b1.reshape(4, C).T)            # (C, 4)
    w2t = np.ascontiguousarray(w2.T.reshape(4, C, HID).transpose(1, 0, 2))
    b2c = np.ascontiguousarray(b2.reshape(4, C).T)
    pt = np.ascontiguousarray(prompt.T.reshape(4, C, K).transpose(1, 0, 2))
    kw = np.ascontiguousarray(kwt.transpose(2, 0, 3, 4, 1).reshape(C, K, KS * KS, C))
    kb = np.ascontiguousarray(kbt)

    in_maps = []
    for c in range(NCORES):
        xs = x[c * BL : (c + 1) * BL]                          # (4, C, H, W)
        xpad = np.zeros((BL, C, HP, WP), np.float32)
        xpad[:, :, 1 : H + 1, 1 : W + 1] = xs
        xpad = round_tf32(xpad).reshape(BL, C, NPIX)
        in_maps.append(
            {
                "xp": xpad, "w1t": w1t, "b1c": b1c, "w2t": w2t, "b2c": b2c,
                "pt": pt, "kb": kb, "kw": kw,
            }
        )
    return in_maps


def kernel(**inputs) -> np.ndarray:
    nc = _get_nc()
    in_maps = prep_inputs(**inputs)
    res = run_bass_kernel_spmd(nc, in_maps, core_ids=list(range(NCORES)))
    outs = [res.results[c]["out"].reshape(BL, C, H, W) for c in range(NCORES)]
    return np.concatenate(outs, axis=0)


if __name__ == "__main__":
    import reference

    inputs = {k: np.asarray(v) for k, v in reference.setup_inputs().items()}
    expected = np.asarray(reference.reference(**inputs))
    actual = kernel(**inputs)
    scale = np.abs(expected).max()
    err = np.abs(actual - expected).max()
    print(f"absmax={err:.3e} scale={scale:.3f} rel={err / scale:.3e}")



# revision 17
# speedup vs baseline: 1.1988x; 1.1988x over previous
"""DynamicConvolution TRN2 Bass kernel (v4 — fully pipelined, bf16).

Problem (per reference):
  x: (32, 128, 64, 64) f32
  attention: pooled = mean(x, HW) -> MLP (relu) -> prompt dot -> softmax over K=8
  agg_w[b] = sum_k alpha[b,k] * kernels_weights[k]  (K=8 banks of (128,128,3,3))
  out[b] = conv2d(x[b], agg_w[b], pad=1) + agg_b[b]   -> (32, 128, 64, 64)

Strategy:
  - Data-parallel over batch: 8 cores x 4 samples.
  - Conv as 9 shifted bf16 matmuls (tap-wise) accumulating in PSUM at
    1 col/cycle; x zero-padded to 66x66 + bf16 on host, so every matmul is a
    full (128 x 512) tile (8 image rows).  fp32 PSUM accumulation.
  - Per-sample attention MLP in fp32; alphas broadcast/transposed on-chip
    with tiny matmuls (no DRAM bounce).  Sample s+1's MLP is burst-split
    into sample s's conv chunk stream so no engine FIFO head-of-line blocks.
  - Kernel bank shipped bf16; per-sample weighted bank sum on DVE all-bf16
    (2x mode), streaming with the per-bank kw DMA arrivals.
  - pooled sums: sample 0 on DVE in DMA-quarters (critical path); samples
    1-3 on the otherwise-idle GpSimd engine.
  - Per-chunk output stores (8 rows each) kill the store tail.
"""
import sys

sys.path.insert(0, "/opt/trn_rl_repo")

import numpy as np
import ml_dtypes

import concourse.bacc as bacc
import concourse.mybir as mybir
import concourse.tile as tile
from concourse.bass_utils import run_bass_kernel_spmd

# problem dims
B, C, H, W = 32, 128, 64, 64
K, KS = 8, 3
HID = 512
NCORES = 8
BL = B // NCORES          # local batch = 4
HP, WP = H + 2, W + 2     # 66x66 padded
NPIX = HP * WP            # 4356
RCHUNK = 8                # output rows per PSUM chunk
NCHUNK = H // RCHUNK      # 8
QROWS = (17, 17, 16, 16)  # x0 DMA quarter row-splits
F32 = mybir.dt.float32
BF16 = mybir.dt.bfloat16
AX = mybir.AxisListType
OP = mybir.AluOpType
AF = mybir.ActivationFunctionType


def build(timing_chain: bool = False, probe_skip=()):
    nc = bacc.Bacc("TRN2", target_bir_lowering=False, debug=False)

    if timing_chain:
        nc.dram_tensor("chain", [BL, C, H * W], F32, kind="ExternalInput")
    xp = nc.dram_tensor("xp", [BL, C, NPIX], BF16, kind="ExternalInput")
    w1t = nc.dram_tensor("w1t", [C, HID], F32, kind="ExternalInput")
    b1c = nc.dram_tensor("b1c", [C, 4], F32, kind="ExternalInput")
    w2t = nc.dram_tensor("w2t", [C, 4, HID], F32, kind="ExternalInput")
    b2c = nc.dram_tensor("b2c", [C, 4], F32, kind="ExternalInput")
    pt = nc.dram_tensor("pt", [C, 4, K], F32, kind="ExternalInput")
    kb = nc.dram_tensor("kb", [K, C], F32, kind="ExternalInput")
    kw = nc.dram_tensor("kw", [C, K, KS * KS, C], BF16, kind="ExternalInput")
    out = nc.dram_tensor("out", [BL, C, H * W], F32, kind="ExternalOutput")

    taps = [(ti, tj) for ti in range(KS) for tj in range(KS)]

    with tile.TileContext(nc) as tc:
        with (
            tc.tile_pool(name="singles", bufs=1) as singles,
            tc.tile_pool(name="xpool", bufs=BL) as xpool,
            tc.tile_pool(name="opool", bufs=4) as opool,
            tc.tile_pool(name="aggpool", bufs=2) as aggpool,
            tc.tile_pool(name="accpool", bufs=1) as accpool,
            tc.tile_pool(name="scr", bufs=2) as scr,
            tc.tile_pool(name="mlpp", bufs=2, space="PSUM") as mlpp,
            tc.tile_pool(name="convp", bufs=4, space="PSUM") as convp,
            tc.tile_pool(name="warmp", bufs=1, space="PSUM") as warmp,
        ):
            # ---- DMAs in priority order (SP queue = issue order) ----
            x_sb = []
            for _ in range(BL):
                xt = xpool.tile([C, HP, WP], BF16, tag="x")
                x_sb.append(xt)
            # sample 0 in four quarters so its pooled reduce overlaps the DMA
            r0 = 0
            for q, qr in enumerate(QROWS):
                nc.sync.dma_start(
                    out=x_sb[0][:, r0 : r0 + qr, :],
                    in_=xp.ap()[0][:, r0 * WP : (r0 + qr) * WP].rearrange(
                        "p (a b) -> p a b", a=qr
                    ),
                )
                r0 += qr
            w1t_sb = singles.tile([C, HID], F32)
            nc.sync.dma_start(out=w1t_sb, in_=w1t.ap())
            b1_sb = singles.tile([C, 4], F32)
            nc.sync.dma_start(out=b1_sb, in_=b1c.ap())
            w2t_sb = singles.tile([C, 4, HID], F32)
            nc.sync.dma_start(out=w2t_sb, in_=w2t.ap())
            b2_sb = singles.tile([C, 4], F32)
            nc.sync.dma_start(out=b2_sb, in_=b2c.ap())
            pt_sb = singles.tile([C, 4, K], F32)
            nc.sync.dma_start(out=pt_sb, in_=pt.ap())
            kb_sb = singles.tile([K, C], F32)
            nc.sync.dma_start(out=kb_sb, in_=kb.ap())
            # x1-x3 in halves so ACT pooled-accums stream with the DMAs
            for s in (1, 2, 3):
                nc.sync.dma_start(
                    out=x_sb[s][:, 0:HP // 2, :],
                    in_=xp.ap()[s][:, : (HP // 2) * WP].rearrange(
                        "p (a b) -> p a b", a=HP // 2
                    ),
                )
                nc.sync.dma_start(
                    out=x_sb[s][:, HP // 2 :, :],
                    in_=xp.ap()[s][:, (HP // 2) * WP :].rearrange(
                        "p (a b) -> p a b", a=HP - HP // 2
                    ),
                )
            # kernel bank per-k, last: the agg chains stream with arrival and
            # everything the MLPs need lands first
            kw_sb = singles.tile([C, K, KS * KS, C], BF16)
            for k in range(K):
                nc.sync.dma_start(out=kw_sb[:, k], in_=kw.ap()[:, k])

            # ---- consts / persistent tiles ----
            ones128 = singles.tile([1, 128], F32)
            nc.gpsimd.memset(ones128, 1.0)
            one1 = singles.tile([1, 1], F32)
            nc.gpsimd.memset(one1, 1.0)
            pooled = singles.tile([C, BL], F32)
            junk = singles.tile([C, NPIX], BF16)
            ph = singles.tile([C, 10], F32)
            h_sb = singles.tile([C, 4, BL], F32)
            s_sb = singles.tile([C, 4, BL], F32)
            aggb_sb = singles.tile([C, BL], F32)

            def pooled_s(s):
                # mean folded into w1t scale host-side; just sums here
                if s == 0:
                    # quarters on DVE, streaming with the quarter DMAs
                    r0 = 0
                    for q, qr in enumerate(QROWS):
                        nc.vector.tensor_reduce(
                            ph[:, q : q + 1], x_sb[0][:, r0 : r0 + qr, :],
                            axis=AX.XY, op=OP.add,
                        )
                        r0 += qr
                    nc.vector.tensor_reduce(
                        pooled[:, 0:1], ph, axis=AX.X, op=OP.add
                    )
                else:
                    # ACT accumulate per half, streaming with the half DMAs
                    c0, c1 = 4 + 2 * (s - 1), 5 + 2 * (s - 1)
                    nc.scalar.activation(
                        junk[:, : (HP // 2) * WP],
                        x_sb[s][:, 0:HP // 2, :].rearrange("p a b -> p (a b)"),
                        AF.Copy, accum_out=ph[:, c0 : c0 + 1],
                    )
                    nc.scalar.activation(
                        junk[:, (HP // 2) * WP :],
                        x_sb[s][:, HP // 2 :, :].rearrange("p a b -> p (a b)"),
                        AF.Copy, accum_out=ph[:, c1 : c1 + 1],
                    )
                    nc.vector.tensor_reduce(
                        pooled[:, s : s + 1], ph[:, c0 : c1 + 1],
                        axis=AX.X, op=OP.add,
                    )

            def mlp_make(s, state):
                """Per-sample attention MLP as 3 burst stages (so stages can
                interleave into the previous sample's conv chunk stream
                without head-of-line blocking any engine FIFO)."""

                def h_stage():
                    M1 = mlpp.tile([C, 32], F32, tag="mlp")
                    state["M1"] = M1
                    for c in range(4):
                        nc.tensor.matmul(
                            M1[:, c : c + 1], w1t_sb[:, 128 * c : 128 * (c + 1)],
                            pooled[:, s : s + 1], start=True, stop=True,
                        )
                    for c in range(4):
                        nc.vector.tensor_scalar(
                            h_sb[:, c, s : s + 1], M1[:, c : c + 1],
                            b1_sb[:, c : c + 1], 0.0, op0=OP.add, op1=OP.max,
                        )

                def s_stage():
                    M1 = state["M1"]
                    for c2 in range(4):
                        for c in range(4):
                            nc.tensor.matmul(
                                M1[:, 4 + c2 : 5 + c2],
                                w2t_sb[:, c, 128 * c2 : 128 * (c2 + 1)],
                                h_sb[:, c, s : s + 1],
                                start=(c == 0), stop=(c == 3),
                            )
                    for c2 in range(4):
                        nc.vector.tensor_scalar_add(
                            s_sb[:, c2, s : s + 1], M1[:, 4 + c2 : 5 + c2],
                            b2_sb[:, c2 : c2 + 1],
                        )
                    for c2 in range(4):
                        nc.tensor.matmul(
                            M1[0:1, 16:24], s_sb[:, c2, s : s + 1],
                            pt_sb[:, c2, :],
                            start=(c2 == 0), stop=(c2 == 3),
                        )

                def sm_stage():
                    M1 = state["M1"]
                    negmx = scr.tile([1, 1], F32, tag="negmx")
                    nc.vector.tensor_reduce(
                        negmx, M1[0:1, 16:24], axis=AX.X, op=OP.max, negate=True
                    )
                    ex = scr.tile([1, K], F32, tag="ex")
                    nc.scalar.activation(ex, M1[0:1, 16:24], AF.Exp, bias=negmx)
                    sm = scr.tile([1, 1], F32, tag="sm")
                    nc.vector.tensor_reduce(sm, ex, axis=AX.X, op=OP.add)
                    rsm = scr.tile([1, 1], F32, tag="rsm")
                    nc.vector.reciprocal(rsm, sm)
                    alphas = scr.tile([1, K], F32, tag="alphas")
                    nc.vector.tensor_scalar_mul(alphas, ex, rsm)
                    # broadcast alphas to all 128 partitions (PE)
                    nc.tensor.matmul(
                        M1[:, 8:16], ones128, alphas, start=True, stop=True
                    )
                    a_bc = scr.tile([C, K], F32, tag="abc")
                    nc.scalar.copy(a_bc, M1[:, 8:16])
                    # k onto partitions (PE transpose via matmul with ones)
                    nc.tensor.matmul(
                        M1[0:8, 24:25], alphas, one1, start=True, stop=True
                    )
                    ak8 = scr.tile([8, 1], F32, tag="ak8")
                    nc.scalar.copy(ak8, M1[0:8, 24:25])
                    # aggregated bias: kb.T @ alpha
                    nc.tensor.matmul(
                        M1[:, 25:26], kb_sb, ak8, start=True, stop=True
                    )
                    nc.scalar.copy(aggb_sb[:, s : s + 1], M1[:, 25:26])
                    state["abc"] = a_bc

                return h_stage, s_stage, sm_stage

            def agg_s(s, a_bc, prev_aggw):
                """Weighted sum of the 8 kernel banks on DVE, all-bf16: muls
                are tensor_scalar 4x mode (360ns), adds tensor_tensor 2x mode
                (660ns).  Banks 0-5 chain while their DMAs stream; banks 6-7
                pair off-chain so the post-DMA critical path is short.  For
                s>0 the alpha vector is copied through a zero-multiply of the
                previous aggw, serializing the per-sample chains so sample
                0's chain (which gates conv0) owns the DVE."""
                if prev_aggw is not None:
                    gated = scr.tile([C, K], F32, tag="abcg")
                    nc.vector.scalar_tensor_tensor(
                        gated, prev_aggw[:, 0, 0:K], 0.0, a_bc,
                        op0=OP.mult, op1=OP.add,
                    )
                    a_bc = gated
                acc = None
                tmps = []
                for k in range(K - 2):
                    tk = aggpool.tile([C, KS * KS, C], BF16, tag=f"tmp{k % 2}")
                    nc.vector.tensor_scalar_mul(tk, kw_sb[:, k], a_bc[:, k : k + 1])
                    tmps.append(tk)
                    if k == 1:
                        acc = accpool.tile([C, KS * KS, C], BF16, tag="accA")
                        nc.vector.tensor_tensor(acc, tmps[0], tmps[1], op=OP.add)
                    elif k > 1:
                        nxt = accpool.tile(
                            [C, KS * KS, C], BF16,
                            tag="accA" if k % 2 else "accB",
                        )
                        nc.vector.tensor_tensor(nxt, acc, tk, op=OP.add)
                        acc = nxt
                t6 = aggpool.tile([C, KS * KS, C], BF16, tag="tmp0")
                nc.vector.tensor_scalar_mul(t6, kw_sb[:, K - 2], a_bc[:, K - 2 : K - 1])
                t7 = aggpool.tile([C, KS * KS, C], BF16, tag="tmp1")
                nc.vector.tensor_scalar_mul(t7, kw_sb[:, K - 1], a_bc[:, K - 1 : K])
                t67 = accpool.tile([C, KS * KS, C], BF16, tag="accB")
                nc.vector.tensor_tensor(t67, t6, t7, op=OP.add)
                aggw = aggpool.tile([C, KS * KS, C], BF16, tag="aggw")
                nc.vector.tensor_tensor(aggw, acc, t67, op=OP.add)
                return aggw

            def conv_s(s, aggw, hooks=None):
                for chunk in range(NCHUNK):
                    h0 = chunk * RCHUNK
                    ps_c = convp.tile([C, RCHUNK, W], F32, tag="ps_c")
                    for t, (ti, tj) in enumerate(taps):
                        nc.tensor.matmul(
                            ps_c, aggw[:, t, :],
                            x_sb[s][:, h0 + ti : h0 + ti + RCHUNK, tj : tj + W],
                            start=(t == 0), stop=(t == KS * KS - 1),
                        )
                    oc = opool.tile([C, RCHUNK, W], F32, tag="oc")
                    nc.scalar.activation(
                        oc, ps_c, AF.Identity, bias=aggb_sb[:, s : s + 1]
                    )
                    nc.sync.dma_start(
                        out=out.ap()[s][:, h0 * W : (h0 + RCHUNK) * W],
                        in_=oc.rearrange("p a b -> p (a b)"),
                    )
                    if hooks and chunk in hooks:
                        hooks[chunk]()

            # ---- interleaved schedule (engine FIFOs follow program order) ----
            # PE warmup: throwaway matmuls keep the PE clock-gate hot
            # through the prologue so the conv stream starts at full rate.
            wlhs = x_sb[0].rearrange("p a b -> p (a b)")[:, 0:128]
            wps = warmp.tile([C, 8, W], F32, tag="warm")
            # first dummy gated on the tail of x3's DMA (~ when the real MLP
            # matmuls have drained) so the WAW-chained dummies only fill idle
            nc.tensor.matmul(
                wps, wlhs, x_sb[3][:, HP - 8 :, 1 : 1 + W], start=True, stop=True
            )
            for i in range(28):
                nc.tensor.matmul(
                    wps, wlhs, x_sb[0][:, (i % 56) : (i % 56) + 8, 1 : 1 + W],
                    start=True, stop=True,
                )
            for k in range(K):
                nc.tensor.matmul(
                    wps, wlhs, kw_sb[:, k, 0:4, :], start=True, stop=True
                )

            # Prologue: all four alpha pipelines complete before aggw0
            # does, so the scheduler's readiness order puts every MLP op
            # ahead of the conv streams on each engine.
            sts = [{}, {}, {}, {}]
            aggws = []
            for s_ in range(BL):
                pooled_s(s_)
                hs, ss, sms = mlp_make(s_, sts[s_])
                hs(); ss(); sms()
                aggws.append(
                    agg_s(s_, sts[s_]["abc"], aggws[-1] if aggws else None)
                )
            for s_ in range(BL):
                conv_s(s_, aggws[s_])

    nc.compile()
    return nc


_NC = None


def _get_nc():
    global _NC
    if _NC is None:
        _NC = build()
    return _NC


def prep_inputs(x, prompt_param, w1, b1, w2, b2, kernels_weights, kernels_bias):
    """Host-side layout transforms -> per-core in_maps."""
    x = np.asarray(x, np.float32)
    prompt = np.asarray(prompt_param, np.float32)[0]          # (K, HID)
    w1 = np.asarray(w1, np.float32)
    b1 = np.asarray(b1, np.float32)
    w2 = np.asarray(w2, np.float32)
    b2 = np.asarray(b2, np.float32)
    kwt = np.asarray(kernels_weights, np.float32)             # (K, C, C, 3, 3)
    kbt = np.asarray(kernels_bias, np.float32)                # (K, C)

    w1t = np.ascontiguousarray(w1.T) * np.float32(1.0 / (H * W))  # (C, HID)
    b1c = np.ascontiguousarray(b1.reshape(4, C).T)            # (C, 4)
    w2t = np.ascontiguousarray(w2.T.reshape(4, C, HID).transpose(1, 0, 2))
    b2c = np.ascontiguousarray(b2.reshape(4, C).T)
    pt = np.ascontiguousarray(prompt.T.reshape(4, C, K).transpose(1, 0, 2))
    kwb = np.ascontiguousarray(
        kwt.transpose(2, 0, 3, 4, 1).reshape(C, K, KS * KS, C)
    ).astype(ml_dtypes.bfloat16)
    kb = np.ascontiguousarray(kbt)

    in_maps = []
    for c in range(NCORES):
        xs = x[c * BL : (c + 1) * BL]                          # (4, C, H, W)
        xpad = np.zeros((BL, C, HP, WP), ml_dtypes.bfloat16)
        xpad[:, :, 1 : H + 1, 1 : W + 1] = xs.astype(ml_dtypes.bfloat16)
        xpad = xpad.reshape(BL, C, NPIX)
        in_maps.append(
            {
                "xp": xpad, "w1t": w1t, "b1c": b1c, "w2t": w2t, "b2c": b2c,
                "pt": pt, "kb": kb, "kw": kwb,
            }
        )
    return in_maps


def kernel(**inputs) -> np.ndarray:
    nc = _get_nc()
    in_maps = prep_inputs(**inputs)
    res = run_bass_kernel_spmd(nc, in_maps, core_ids=list(range(NCORES)))
    outs = [res.results[c]["out"].reshape(BL, C, H, W) for c in range(NCORES)]
    return np.concatenate(outs, axis=0)


if __name__ == "__main__":
    import reference

    inputs = {k: np.asarray(v) for k, v in reference.setup_inputs().items()}
    expected = np.asarray(reference.reference(**inputs))
    actual = kernel(**inputs)
    scale = np.abs(expected).max()
    err = np.abs(actual - expected).max()
    print(f"absmax={err:.3e} scale={scale:.3f} rel={err / scale:.3e}")
